# revision 38
# baseline (speedup 1.0000x reference)
# Bass/Tile Trainium2 kernel for batched multi-head attention with boolean mask.
#
# Problem: q,k,v [B=4, H=16, S=2048, D=128] f32, mask [B, 1, S, S] bool.
#   out = softmax(q@k^T/sqrt(D) + mask*-1e9) @ v
#
# Sharding: 64 (b,h) pairs -> 8 cores x 8 pairs (core c gets batch b=c//2,
# heads (c%2)*8..+8). Each core is fully independent (no collectives).
#
# v4: single-op bf16-bits Schraudolph on DVE (affine_then_add -> i16 bits
# written straight into the bf16 em tile) with the mask folded in via the
# host-built Bnm = 2^14*(1-mask) tensor; 6 of 32 tiles/pair leave ACT (the
# 243us bottleneck) at only ~1.15us DVE each. Engine budget/core: ACT 211,
# DVE 216, PE 170.
#
# v3 design ("S^T layout", host-marshalled), ~277us vs 412us v1 baseline:
#   - HOST pre-transposes + pre-casts q,k -> qT,kT [D,S] bf16, casts v to
#     bf16, and pre-builds nmT = (1-mask)^T bf16 in the exact SBUF tile
#     layout. This removes the entire v1 on-device prologue (256 PE mask
#     transposes + u8 casts + PSUM evictions, ~80us) AND the per-pair
#     q/k DVE casts + 32 PE transposes + evictions. All device input
#     loads are plain contiguous DMAs, chunked so the first consumer of
#     each slice only waits on that slice; pair-0 chunks are issued in
#     critical-path order across both HWDGE queues (sync + scalar).
#   - S^T[kv, q] = matmul(lhsT=kT_tile, rhs=qT_chunk) into PSUM (f32);
#     steady state is stream-bound: QK 215ns/512-col MM, EV 56ns/129-col
#     MM (LDWEIGHTS hidden by bf16 FWL + dual weight buffer).
#   - E^T = exp(S^T/sqrt(D)): ACT (1 el/cycle/lane, the co-bottleneck)
#     for most (qc,kp) tiles; 2 tiles/pair use the Schraudolph bit-trick
#     exp on DVE (i32(A*y+B) bitcast to f32, max rel err 3%) to keep
#     ACT off the critical path. Then DVE multiply by nmT (masked -> 0).
#   - O'[q,0:128] + rowsum[q] in col 128 accumulate in PSUM via
#     matmul(lhsT=E^T tile, rhs=[V | ones]) over kv tiles. Two q-subtile
#     outputs pack per PSUM bank at 520B (8B-aligned) stride, so o_ps
#     takes 2 banks and st2 triple-buffers in 6 (full 8-bank budget).
#     `start` only on the first qs of a shared bank (start clears
#     has_written for the WHOLE bank).
#   - O = O' * reciprocal(rowsum): one [128,2,1] reciprocal + one
#     broadcast tensor_tensor per bank (batched, 37us vs 63us on DVE).
#   - ACT exp-table load + PE HAM clock-gate warm-up run during the DMA
#     ramp.
# Softmax max-subtraction is skipped: scores/sqrt(D) ~ N(0,1), |s|<=sqrt(D)
# so exp never overflows f32; masked lanes are exactly 0 both ways.
# Engine budget/core: PE 243us (stream floor 225), ACT 250, DVE 220.
# Rel err 9.9e-3 (gate 2e-2), dominated by the 16 Schraudolph tiles.
# NOTE: fp8 was tried and rejected (DVE TT 2x slower at 8-bit, DoubleRow
# disables FWL at FD=129); GpSimd offload rejected (TT 4x slower + SBUF
# port conflict throttles DVE).

import os
import sys
import types

# Reset cores at runtime init: recovers the chip from a stuck P0 power-state
# downclock (2.0 GHz instead of 2.4 — observed as a persistent uniform ~1.2x
# slowdown across all engines). Must be set before the neuron runtime loads.
os.environ.setdefault("NEURON_RT_RESET_CORES", "1")

import numpy as np

if "/opt/trn_rl_repo" not in sys.path:
    sys.path.insert(0, "/opt/trn_rl_repo")

import concourse.bass as bass
import concourse.tile as tile
from concourse import bacc, mybir

B, H, S_FULL, D = 4, 16, 2048, 128
N_CORES = 8
PAIRS = (B * H) // N_CORES  # 8

F32 = mybir.dt.float32
BF16 = mybir.dt.bfloat16
FP8 = mybir.dt.float8e4


def _install_ntff_hook():
    """Best-effort: register the axon NTFF profile hook missing from this
    image's antenv so run_bass_kernel_spmd(trace=True) can profile."""
    try:
        import antenv

        if "antenv.axon_hooks" in sys.modules:
            return
        mod = types.ModuleType("antenv.axon_hooks")
        mod._hook = None
        mod.set_axon_ntff_profile_hook = lambda h: setattr(mod, "_hook", h)
        mod.get_axon_ntff_profile_hook = lambda: mod._hook
        sys.modules["antenv.axon_hooks"] = mod
        antenv.axon_hooks = mod
        from trn_agent_boot.trn_boot import _ntff_profile_via_ctypes

        mod._hook = _ntff_profile_via_ctypes("/opt/axon/libaxon_pjrt.so")
    except Exception:
        pass


def build_nc(S=S_FULL, pairs=PAIRS, split=True):
    """Build the per-core Bass module. S must be a multiple of 512.
    split=True runs Bacc.compile (multi-wait splitting for hardware)."""
    assert S % 512 == 0
    T = S // 128  # 128-row tiles along seq
    QCW = 512  # q-chunk width
    NQC = S // QCW
    NQS = QCW // 128  # q-subtiles per chunk
    KP = T // 2  # kv tile pairs
    scale = float(np.float32(1.0) / np.sqrt(np.float32(D)))

    nc = bacc.Bacc("TRN2", target_bir_lowering=False, debug=False)
    # q, k arrive host-pre-transposed: [pairs, D, S]
    q_d = nc.dram_tensor("q", [pairs, D, S], BF16, kind="ExternalInput").ap()
    k_d = nc.dram_tensor("k", [pairs, D, S], BF16, kind="ExternalInput").ap()
    v_d = nc.dram_tensor("v", [pairs, S, D], BF16, kind="ExternalInput").ap()
    # host-prebuilt (1-mask)^T in SBUF tile layout [kv%128, qc, kp, h, qw]
    m_d = nc.dram_tensor(
        "mask", [128, NQC, KP, 2, QCW], BF16, kind="ExternalInput"
    ).ap()
    # 0.0625*I for the Madd slots' PE mask-accumulate
    i_d = nc.dram_tensor("ident", [128, 128], BF16, kind="ExternalInput").ap()
    o_d = nc.dram_tensor("o", [pairs, S, D], F32, kind="ExternalOutput").ap()

    Exp = mybir.ActivationFunctionType.Exp
    mult = mybir.AluOpType.mult
    add = mybir.AluOpType.add

    # v4: bf16-bits Schraudolph on DVE — ONE fused op per offloaded tile:
    #   em_bits[i16] = i16(A16*s + C16 + Bnm)   (affine_then_add, in1=mask)
    # where Bnm = 2^14*(1-mask) in bf16 (host-built). Unmasked lanes get
    # bits ~ 128*(141 + log2(e)*scale*s) -> reinterpreted as bf16 this IS
    # 2^14*exp(scale*s)*(1±3.3%); masked lanes get bits ~1.7e3 -> 2^-105 ~ 0.
    # The 2^14 factor also multiplies the ACT-path tiles (em = e2 * Bnm), so
    # softmax normalization cancels it exactly. This removes the separate
    # i32 tensor_scalar + f32 mask-multiply (2.3us/slot -> 1.15us/slot DVE)
    # and lets 6 of 32 tiles/pair leave the ACT engine (the bottleneck).
    SCHRAU_A16 = (2.0**7 / float(np.log(2.0))) * scale
    # 128*(127+14) - centering (366400/2^23 in octaves, = 5.591 bits-lsb),
    # minus the 2^14 carried by the Bnm tensor itself.
    SCHRAU_C16 = 128.0 * 141.0 - 366400.0 / 65536.0 - 16384.0
    # Slot types per (qc, kp), scattered so consecutive st2 banks mix ACT-
    # and DVE-readers (a clustered run of one engine's readers stalls the
    # 3-deep st2 PSUM rotation behind that engine's in-order queue).
    # n=10 schrau / k=10 madd / 12 TT: ACT 187 / DVE 186 / PE 190 us/core.
    schrau_set = {(0, 2), (0, 4), (0, 6), (1, 2), (1, 4),
                  (2, 2), (2, 4), (3, 2), (3, 4), (3, 6)}
    # "Madd" slots: the mask lands in the PSUM scores via one extra PE matmul
    # per h (lhsT = 0.0625*I, rhs = Bnm -> adds 1024*(1-m)), and ACT's free
    # bias turns exp(scale*s + 14*ln2 - 1024*scale + 1024*scale*(1-m)) into
    # 2^14*exp(scale*s) unmasked / ~e^-80 ~ 0 masked. No DVE op at all.
    MADD_BIAS = 14.0 * float(np.log(2.0)) - 1024.0 * scale
    madd_set = {(0, 1), (0, 5), (1, 1), (1, 5), (1, 6),
                (2, 1), (2, 5), (2, 6), (3, 1), (3, 5)}

    with tile.TileContext(nc) as tc:
        from contextlib import ExitStack

        with ExitStack() as ctx:
            nmT_pool = ctx.enter_context(tc.tile_pool(name="nmTp", bufs=1))
            psum_pool = ctx.enter_context(
                tc.tile_pool(name="psum", bufs=2, space="PSUM")
            )
            qkv_pool = ctx.enter_context(tc.tile_pool(name="qkv", bufs=2))
            e_pool = ctx.enter_context(tc.tile_pool(name="e", bufs=2))
            out_pool = ctx.enter_context(tc.tile_pool(name="outp", bufs=2))

            # Warm the ACT exp table (~2.7us PSEUDO_LOAD_ACT_FUNC_SET) during
            # the DMA ramp instead of stalling the first real exp.
            warm = out_pool.tile([128, 1], F32, name="act_warm", tag="warm")
            nc.vector.memset(warm[:], 0.0)
            nc.scalar.activation(warm[:], warm[:], Exp)
            # per-partition bias AP for the Madd slots' exp
            madd_bias = out_pool.tile([128, 1], F32, name="madd_bias", tag="mb")
            nc.gpsimd.memset(madd_bias[:], MADD_BIAS)
            # Warm the PE HAM clock gate during the DMA ramp: ~4us of dummy
            # matmuls flips the PE from K=4/8 (1.2 GHz) to 8/8 (2.4 GHz) so
            # the first real QKs run at full rate.
            warm_mm = out_pool.tile([128, 128], BF16, name="warm_mm", tag="wmm")
            nc.vector.memset(warm_mm[:], 0.0)
            warm_ps = psum_pool.tile(
                [128, 2, QCW], F32, name="warm_ps", tag="ps", bufs=3
            )
            for i in range(40):
                nc.tensor.matmul(
                    warm_ps[:, 0, 0:128],
                    lhsT=warm_mm[:],
                    rhs=warm_mm[:],
                    start=True,
                    stop=True,
                )

            # (1 - mask) transposed, contiguous per (qc, kp):
            # nmT[qc][kv%128, kp, h, qw] = 1 - mask[qc*512+qw, (2kp+h)*128+kv%128]
            # One tile per (qc, kp-half) so the first pair's TT only waits on
            # the chunk it reads, not the whole 8MB mask load.
            nmT_t = {}

            def load_nmT(qc, kh, split=1):
                t = nmT_pool.tile(
                    [128, KP // 2, 2, QCW], BF16, name=f"nmT_{qc}_{kh}"
                )
                k0 = kh * (KP // 2)
                for j in range(split):
                    w = (KP // 2) // split
                    nc.sync.dma_start(
                        t[:, j * w : (j + 1) * w],
                        m_d[:, qc, k0 + j * w : k0 + (j + 1) * w],
                    )
                nmT_t[(qc, kh)] = t

            def nmT(qc, kp):
                return nmT_t[(qc, kp // (KP // 2))][:, kp % (KP // 2)]

            # kT/qT split into chunk tiles so the first QK only waits on
            # the slice it reads (cuts the pipeline ramp at start).
            def load_kTh(p, h, eng=None):
                t = qkv_pool.tile(
                    [128, S // 2], BF16, name=f"kT_{p}_{h}", tag=f"kT{h}"
                )
                (eng or nc.sync).dma_start(
                    t[:], k_d[p, :, h * (S // 2) : (h + 1) * (S // 2)]
                )
                return t

            def load_qTc(p, c, eng=None):
                t = qkv_pool.tile(
                    [128, QCW], BF16, name=f"qT_{p}_{c}", tag=f"qT{c}"
                )
                (eng or nc.sync).dma_start(t[:], q_d[p, :, c * QCW : (c + 1) * QCW])
                return t

            def load_vb(p, eng=None):
                vb = qkv_pool.tile([128, T, D + 1], BF16, name=f"vb_{p}", tag="vb")
                (eng or nc.sync).dma_start(
                    vb[:, :, 0:D], v_d[p].rearrange("(t p) d -> p t d", p=128)
                )
                nc.gpsimd.memset(vb[:, :, D : D + 1], 1.0)
                return vb

            def load_pair(p):
                kTh = [load_kTh(p, h) for h in range(2)]
                qTc = [load_qTc(p, c) for c in range(NQC)]
                return qTc, kTh, load_vb(p)

            # Pair-0 DMAs issued in critical-path order: first QK needs
            # kT cols 0:128 + qT chunk 0; first TT needs mask chunk (0,0);
            # first EV needs vb. qT c0 goes first on sync (128KB) while kT
            # half 0 lands in four 256-col pieces on the scalar queue so the
            # first QK only waits on ~64KB there, not the whole 256KB tile.
            q0c0 = load_qTc(0, 0)
            k0h0 = qkv_pool.tile([128, S // 2], BF16, name="kT_0_0", tag="kT0")
            for j in range(4):
                nc.scalar.dma_start(
                    k0h0[:, j * 256 : (j + 1) * 256],
                    k_d[0, :, j * 256 : (j + 1) * 256],
                )
            vb0 = load_vb(0, eng=nc.scalar)
            load_nmT(0, 0, split=4)
            idt = nmT_pool.tile([128, 128], BF16, name="ident")
            nc.scalar.dma_start(idt[:], i_d[:, :])
            k0h1 = load_kTh(0, 1, eng=nc.scalar)
            q0rest = [load_qTc(0, c) for c in range(1, NQC)]
            loads = {0: ([q0c0] + q0rest, [k0h0, k0h1], vb0)}
            loads[1] = load_pair(1)
            load_nmT(0, 1)
            for qc in range(1, NQC):
                load_nmT(qc, 0)
                load_nmT(qc, 1)

            # Software-pipelined qc schedule. Each qc runs in two passes:
            #   P1(g):  QK (+Madd) matmuls, schrau affines, and exps — every
            #           op that READS an st2 PSUM bank, with no cross-engine
            #           producer in front of it. Bank release is paced by
            #           ACT's exp stream and the DVE affines at queue head.
            #   P2(g):  the TT mask-multiplies, EV matmuls, and the norm of
            #           g-1 — emitted AFTER P1(g+1), so on the DVE queue the
            #           next qc's affines always jump ahead of older TTs, and
            #           on the PE queue EVs never sit in front of QKs.
            # This breaks the convoy (aff waits TTs waits exps) that cost
            # ~2.4us/qc in the slot-interleaved schedule.
            step_prev = None  # per-slot deferred TT/EV of the previous qc
            fin_flush = None  # flushes prev qc's last EV + prev-prev's norm
            fin_prev = None   # norm+DMA closure of the lagged qc

            for p in range(pairs):
                qTc, kTh, vb = loads.pop(p)

                def kT_ap(kt, kTh=kTh):
                    return kTh[kt // (T // 2)][
                        :, (kt % (T // 2)) * 128 : (kt % (T // 2) + 1) * 128
                    ]
                if p + 2 < pairs:
                    loads[p + 2] = load_pair(p + 2)

                o_re = o_d[p].rearrange("(t p) d -> p t d", p=128)
                for qc in range(NQC):
                    # Two q-subtile outputs packed per PSUM bank (1040B,
                    # 8B-aligned 520B stride) so o_ps takes 2 banks total.
                    # Allocation is LAZY (first EV emission, in P2): the pool
                    # snapshots hazards at allocation, and the previous
                    # user's norm reads are emitted two P2s earlier.
                    o_cell = {}

                    def o_ps2(p=p, qc=qc, o_cell=o_cell):
                        if "t" not in o_cell:
                            o_cell["t"] = [
                                psum_pool.tile(
                                    [128, 2, D + 2], F32,
                                    name=f"ops_{p}_{qc}_{j}", tag="ops", bufs=2,
                                )
                                for j in range(NQS // 2)
                            ]
                        return o_cell["t"]

                    def o_ap(qs, o_ps2=o_ps2):
                        # [*, qs, 0:129] window; col 129 is padding.
                        return o_ps2()[qs // 2][:, qs % 2, 0 : D + 1]

                    def emit_ev(kp, em, o_ap=o_ap, vb=vb):
                        for h in (0, 1):
                            kt = 2 * kp + h
                            for qs in range(NQS):
                                # start clears has_written for the WHOLE bank;
                                # only the first qs of each shared bank may set
                                # it (the odd qs then overwrites its still-
                                # cleared region).
                                nc.tensor.matmul(
                                    o_ap(qs),
                                    lhsT=em[:, h, qs * 128 : (qs + 1) * 128],
                                    rhs=vb[:, kt, :],
                                    start=(kt == 0 and qs % 2 == 0),
                                    stop=(kt == T - 1),
                                    skip_group_check=True,
                                )

                    def finish_qc(o_ps2=o_ps2, p=p, qc=qc, o_re=o_re):
                        ot = o_ps2()
                        osb = out_pool.tile(
                            [128, NQS, D], F32, name=f"osb_{p}_{qc}", tag="osb"
                        )
                        for j in range(NQS // 2):
                            # batched: one reciprocal + one broadcast-multiply
                            # per PSUM bank (both packed q-subtiles at once)
                            rs2 = out_pool.tile(
                                [128, 2, 1], F32, name=f"rs_{p}_{qc}_{j}",
                                tag="rs", bufs=4,
                            )
                            nc.vector.reciprocal(
                                rs2[:], ot[j][:, :, D : D + 1]
                            )
                            nc.vector.tensor_tensor(
                                osb[:, 2 * j : 2 * j + 2, :],
                                ot[j][:, :, 0:D],
                                rs2[:].broadcast_to([128, 2, D]),
                                mult,
                            )
                        nc.sync.dma_start(
                            o_re[:, qc * NQS : (qc + 1) * NQS, :], osb[:]
                        )

                    # ---- P1: QK/Madd matmuls + bank-releasing reads,
                    #      interleaved with the PREVIOUS qc's deferred
                    #      TT/EV work (one slot's worth per kp) ----
                    ems = {}   # kp -> em tile (schrau/madd: final; TT: later)
                    e2s = {}   # kp -> e2 tile for TT slots
                    for kp in range(KP):
                        is_madd = (qc, kp) in madd_set
                        st2 = psum_pool.tile(
                            [128, 2, QCW], F32, name=f"st_{p}_{qc}_{kp}",
                            tag="ps", bufs=3,
                        )
                        for h in (0, 1):
                            nc.tensor.matmul(
                                st2[:, h, :],
                                lhsT=kT_ap(2 * kp + h),
                                rhs=qTc[qc][:],
                                start=True,
                                stop=not is_madd,
                            )
                            if is_madd:
                                nc.tensor.matmul(
                                    st2[:, h, :],
                                    lhsT=idt[:],
                                    rhs=nmT(qc, kp)[:, h],
                                    start=False,
                                    stop=True,
                                    skip_group_check=True,
                                )
                        if (qc, kp) in schrau_set:
                            em = e_pool.tile(
                                [128, 2, QCW], BF16, name=f"em_{p}_{qc}_{kp}",
                                tag="em", bufs=16,
                            )
                            # single DVE op: bf16-bits Schraudolph with the
                            # mask folded in via the Bnm (=2^14*(1-m)) add
                            nc.vector.affine_then_add(
                                em[:].bitcast(mybir.dt.int16),
                                st2[:],
                                nmT(qc, kp),
                                SCHRAU_A16,
                                SCHRAU_C16,
                            )
                            ems[kp] = em
                        elif is_madd:
                            em = e_pool.tile(
                                [128, 2, QCW], BF16, name=f"em_{p}_{qc}_{kp}",
                                tag="em", bufs=16,
                            )
                            # mask already in the scores; bias closes the
                            # 2^14 bookkeeping. No DVE op.
                            nc.scalar.activation(
                                em[:], st2[:], Exp, scale=scale,
                                bias=madd_bias[:],
                            )
                            ems[kp] = em
                        else:
                            e2 = e_pool.tile(
                                [128, 2, QCW], BF16, name=f"e_{p}_{qc}_{kp}",
                                tag="e2", bufs=12,
                            )
                            nc.scalar.activation(e2[:], st2[:], Exp, scale=scale)
                            e2s[kp] = e2
                        if step_prev is not None:
                            step_prev(kp)
                    if step_prev is not None:
                        fin_prev = fin_flush(fin_prev)

                    def step_prev(kp, p=p, qc=qc, ems=ems, e2s=e2s,
                                  emit_ev=emit_ev):
                        # one deferred slot of THIS qc, run during the next
                        # qc's P1: the TT mask-multiply for slot kp and the
                        # EV batch for slot kp-1 (whose em is ready).
                        if kp in e2s:
                            em = e_pool.tile(
                                [128, 2, QCW], BF16,
                                name=f"em_{p}_{qc}_{kp}", tag="em", bufs=16,
                            )
                            nc.vector.tensor_tensor(
                                em[:], e2s[kp][:], nmT(qc, kp), mult
                            )
                            ems[kp] = em
                        if kp >= 1:
                            emit_ev(kp - 1, ems[kp - 1])

                    def fin_flush(fin_before, ems=ems, emit_ev=emit_ev,
                                  finish_qc=finish_qc):
                        # last EV batch of the lagged qc, then the norm+DMA
                        # of the qc before it (keeps norm behind the affines
                        # and TTs on the DVE queue).
                        emit_ev(KP - 1, ems[KP - 1])
                        if fin_before is not None:
                            fin_before()
                        return finish_qc

            # drain the pipeline tail: deferred TT/EV of the final qc, its
            # predecessor's norm, then the final norm+DMA
            for kp in range(KP):
                step_prev(kp)
            fin_prev = fin_flush(fin_prev)
            fin_prev()

    if split:
        nc.compile()
    return nc


_NC_CACHE = {}


def _get_nc(S=S_FULL, pairs=PAIRS):
    key = (S, pairs)
    if key not in _NC_CACHE:
        _NC_CACHE[key] = build_nc(S, pairs)
    return _NC_CACHE[key]


def kernel(q, k, v, mask):
    """Full-input entry point: q,k,v [4,16,2048,128] f32, mask [4,1,2048,2048]
    bool. Returns [4,16,2048,128] f32."""
    _install_ntff_hook()
    import ml_dtypes
    from concourse.bass_utils import run_bass_kernel_spmd

    bf16 = ml_dtypes.bfloat16
    # q, k pre-transposed per head: [B, H, D, S]; v kept [B, H, S, D]
    q = np.asarray(q, dtype=np.float32).transpose(0, 1, 3, 2).astype(bf16)
    k = np.asarray(k, dtype=np.float32).transpose(0, 1, 3, 2).astype(bf16)
    v = np.asarray(v, dtype=np.float32).astype(bf16)

    S = S_FULL
    NQC, KP, QCW = S // 512, (S // 128) // 2, 512
    # Bnm[b][p, qc, kp, h, qw] = 2^14 * (1 - mask[b, 0, qc*512+qw, (2kp+h)*128+p])
    # The 2^14 scale rides both the ACT path (multiplied into em) and the
    # Schraudolph path (added in bits-space); softmax normalization cancels it.
    nm = 16384.0 * (
        1.0 - np.asarray(mask).reshape(B, S, S).astype(np.float32)
    )
    # [b, qc, qw, kp, h, p] -> transpose to [b, p, qc, kp, h, qw]
    nmT = np.ascontiguousarray(
        nm.reshape(B, NQC, QCW, KP, 2, 128).transpose(0, 5, 1, 3, 4, 2)
    ).astype(bf16)

    ident = (0.0625 * np.eye(128, dtype=np.float32)).astype(bf16)
    hpc = H // (N_CORES // B)  # heads per core = 8
    in_maps = []
    for c in range(N_CORES):
        b = c // (N_CORES // B)
        h0 = (c % (N_CORES // B)) * hpc
        in_maps.append(
            {
                "q": np.ascontiguousarray(q[b, h0 : h0 + hpc]),
                "k": np.ascontiguousarray(k[b, h0 : h0 + hpc]),
                "v": np.ascontiguousarray(v[b, h0 : h0 + hpc]),
                "mask": nmT[b],
                "ident": ident,
            }
        )

    nc = _get_nc()
    trace = os.environ.get("BASS_ATTN_TRACE", "0") == "1"
    res = run_bass_kernel_spmd(nc, in_maps, list(range(N_CORES)), trace=trace)
    if trace:
        kernel.last_exec_time_ns = res.exec_time_ns
        kernel.last_results = res

    out = np.empty((B, H, S_FULL, D), dtype=np.float32)
    for c in range(N_CORES):
        b = c // (N_CORES // B)
        h0 = (c % (N_CORES // B)) * hpc
        out[b, h0 : h0 + hpc] = res.results[c]["o"]
    return out



# revision 43
# speedup vs baseline: 1.0657x; 1.0657x over previous
# Bass/Tile Trainium2 kernel for batched multi-head attention with boolean mask.
#
# Problem: q,k,v [B=4, H=16, S=2048, D=128] f32, mask [B, 1, S, S] bool.
#   out = softmax(q@k^T/sqrt(D) + mask*-1e9) @ v
#
# Sharding: 64 (b,h) pairs -> 8 cores x 8 pairs (core c gets batch b=c//2,
# heads (c%2)*8..+8). Each core is fully independent (no collectives).
#
# v4: single-op bf16-bits Schraudolph on DVE (affine_then_add -> i16 bits
# written straight into the bf16 em tile) with the mask folded in via the
# host-built Bnm = 2^14*(1-mask) tensor; 6 of 32 tiles/pair leave ACT (the
# 243us bottleneck) at only ~1.15us DVE each. Engine budget/core: ACT 211,
# DVE 216, PE 170.
#
# v3 design ("S^T layout", host-marshalled), ~277us vs 412us v1 baseline:
#   - HOST pre-transposes + pre-casts q,k -> qT,kT [D,S] bf16, casts v to
#     bf16, and pre-builds nmT = (1-mask)^T bf16 in the exact SBUF tile
#     layout. This removes the entire v1 on-device prologue (256 PE mask
#     transposes + u8 casts + PSUM evictions, ~80us) AND the per-pair
#     q/k DVE casts + 32 PE transposes + evictions. All device input
#     loads are plain contiguous DMAs, chunked so the first consumer of
#     each slice only waits on that slice; pair-0 chunks are issued in
#     critical-path order across both HWDGE queues (sync + scalar).
#   - S^T[kv, q] = matmul(lhsT=kT_tile, rhs=qT_chunk) into PSUM (f32);
#     steady state is stream-bound: QK 215ns/512-col MM, EV 56ns/129-col
#     MM (LDWEIGHTS hidden by bf16 FWL + dual weight buffer).
#   - E^T = exp(S^T/sqrt(D)): ACT (1 el/cycle/lane, the co-bottleneck)
#     for most (qc,kp) tiles; 2 tiles/pair use the Schraudolph bit-trick
#     exp on DVE (i32(A*y+B) bitcast to f32, max rel err 3%) to keep
#     ACT off the critical path. Then DVE multiply by nmT (masked -> 0).
#   - O'[q,0:128] + rowsum[q] in col 128 accumulate in PSUM via
#     matmul(lhsT=E^T tile, rhs=[V | ones]) over kv tiles. Two q-subtile
#     outputs pack per PSUM bank at 520B (8B-aligned) stride, so o_ps
#     takes 2 banks and st2 triple-buffers in 6 (full 8-bank budget).
#     `start` only on the first qs of a shared bank (start clears
#     has_written for the WHOLE bank).
#   - O = O' * reciprocal(rowsum): one [128,2,1] reciprocal + one
#     broadcast tensor_tensor per bank (batched, 37us vs 63us on DVE).
#   - ACT exp-table load + PE HAM clock-gate warm-up run during the DMA
#     ramp.
# Softmax max-subtraction is skipped: scores/sqrt(D) ~ N(0,1), |s|<=sqrt(D)
# so exp never overflows f32; masked lanes are exactly 0 both ways.
# Engine budget/core: PE 243us (stream floor 225), ACT 250, DVE 220.
# Rel err 9.9e-3 (gate 2e-2), dominated by the 16 Schraudolph tiles.
# NOTE: fp8 was tried and rejected (DVE TT 2x slower at 8-bit, DoubleRow
# disables FWL at FD=129); GpSimd offload rejected (TT 4x slower + SBUF
# port conflict throttles DVE).

import os
import sys
import types

# Reset cores at runtime init: recovers the chip from a stuck P0 power-state
# downclock (2.0 GHz instead of 2.4 — observed as a persistent uniform ~1.2x
# slowdown across all engines). Must be set before the neuron runtime loads.
os.environ.setdefault("NEURON_RT_RESET_CORES", "1")

import numpy as np

if "/opt/trn_rl_repo" not in sys.path:
    sys.path.insert(0, "/opt/trn_rl_repo")

import concourse.bass as bass
import concourse.tile as tile
from concourse import bacc, mybir

B, H, S_FULL, D = 4, 16, 2048, 128
N_CORES = 8
PAIRS = (B * H) // N_CORES  # 8

F32 = mybir.dt.float32
BF16 = mybir.dt.bfloat16
FP8 = mybir.dt.float8e4


def _install_ntff_hook():
    """Best-effort: register the axon NTFF profile hook missing from this
    image's antenv so run_bass_kernel_spmd(trace=True) can profile."""
    try:
        import antenv

        if "antenv.axon_hooks" in sys.modules:
            return
        mod = types.ModuleType("antenv.axon_hooks")
        mod._hook = None
        mod.set_axon_ntff_profile_hook = lambda h: setattr(mod, "_hook", h)
        mod.get_axon_ntff_profile_hook = lambda: mod._hook
        sys.modules["antenv.axon_hooks"] = mod
        antenv.axon_hooks = mod
        from trn_agent_boot.trn_boot import _ntff_profile_via_ctypes

        mod._hook = _ntff_profile_via_ctypes("/opt/axon/libaxon_pjrt.so")
    except Exception:
        pass


def build_nc(S=S_FULL, pairs=PAIRS, split=True):
    """Build the per-core Bass module. S must be a multiple of 512.
    split=True runs Bacc.compile (multi-wait splitting for hardware)."""
    assert S % 512 == 0
    T = S // 128  # 128-row tiles along seq
    QCW = 512  # q-chunk width
    NQC = S // QCW
    NQS = QCW // 128  # q-subtiles per chunk
    KP = T // 2  # kv tile pairs
    scale = float(np.float32(1.0) / np.sqrt(np.float32(D)))

    nc = bacc.Bacc("TRN2", target_bir_lowering=False, debug=False)
    # q, k arrive host-pre-transposed: [pairs, D, S]
    q_d = nc.dram_tensor("q", [pairs, D, S], BF16, kind="ExternalInput").ap()
    k_d = nc.dram_tensor("k", [pairs, D, S], BF16, kind="ExternalInput").ap()
    v_d = nc.dram_tensor("v", [pairs, S, D], BF16, kind="ExternalInput").ap()
    # host-prebuilt (1-mask)^T in SBUF tile layout [kv%128, qc, kp, h, qw]
    m_d = nc.dram_tensor(
        "mask", [128, NQC, KP, 2, QCW], BF16, kind="ExternalInput"
    ).ap()
    # 0.0625*I for the Madd slots' PE mask-accumulate
    i_d = nc.dram_tensor("ident", [128, 128], BF16, kind="ExternalInput").ap()
    o_d = nc.dram_tensor("o", [pairs, S, D], F32, kind="ExternalOutput").ap()

    Exp = mybir.ActivationFunctionType.Exp
    mult = mybir.AluOpType.mult
    add = mybir.AluOpType.add

    # v4: bf16-bits Schraudolph on DVE — ONE fused op per offloaded tile:
    #   em_bits[i16] = i16(A16*s + C16 + Bnm)   (affine_then_add, in1=mask)
    # where Bnm = 2^14*(1-mask) in bf16 (host-built). Unmasked lanes get
    # bits ~ 128*(141 + log2(e)*scale*s) -> reinterpreted as bf16 this IS
    # 2^14*exp(scale*s)*(1±3.3%); masked lanes get bits ~1.7e3 -> 2^-105 ~ 0.
    # The 2^14 factor also multiplies the ACT-path tiles (em = e2 * Bnm), so
    # softmax normalization cancels it exactly. This removes the separate
    # i32 tensor_scalar + f32 mask-multiply (2.3us/slot -> 1.15us/slot DVE)
    # and lets 6 of 32 tiles/pair leave the ACT engine (the bottleneck).
    SCHRAU_A16 = (2.0**7 / float(np.log(2.0))) * scale
    # 128*(127+14) - centering (366400/2^23 in octaves, = 5.591 bits-lsb),
    # minus the 2^14 carried by the Bnm tensor itself.
    SCHRAU_C16 = 128.0 * 141.0 - 366400.0 / 65536.0 - 16384.0
    # Slot types per (qc, kp), scattered so consecutive st2 banks mix ACT-
    # and DVE-readers (a clustered run of one engine's readers stalls the
    # 3-deep st2 PSUM rotation behind that engine's in-order queue).
    # Uniform (schrau, madd, TT) = (2, 2, 4) per qc: ACT 205 / DVE 185 /
    # PE 186 us/core. DVE is deliberately ~10% under ACT — when DVE nears
    # saturation its in-order queue delays the affines' st2 reads and the
    # 3-deep bank rotation stalls ACT (measured +2.2us/qc at parity).
    schrau_set = {(qc, kp) for qc in range(4) for kp in (2, 4)}
    # "Madd" slots: the mask lands in the PSUM scores via one extra PE matmul
    # per h (lhsT = 0.0625*I, rhs = Bnm -> adds 1024*(1-m)), and ACT's free
    # bias turns exp(scale*s + 14*ln2 - 1024*scale + 1024*scale*(1-m)) into
    # 2^14*exp(scale*s) unmasked / ~e^-80 ~ 0 masked. No DVE op at all.
    MADD_BIAS = 14.0 * float(np.log(2.0)) - 1024.0 * scale
    madd_set = {(qc, kp) for qc in range(4) for kp in (1, 5)}

    with tile.TileContext(nc) as tc:
        from contextlib import ExitStack

        with ExitStack() as ctx:
            nmT_pool = ctx.enter_context(tc.tile_pool(name="nmTp", bufs=1))
            psum_pool = ctx.enter_context(
                tc.tile_pool(name="psum", bufs=2, space="PSUM")
            )
            qkv_pool = ctx.enter_context(tc.tile_pool(name="qkv", bufs=2))
            e_pool = ctx.enter_context(tc.tile_pool(name="e", bufs=2))
            out_pool = ctx.enter_context(tc.tile_pool(name="outp", bufs=2))

            # Warm the ACT exp table (~2.7us PSEUDO_LOAD_ACT_FUNC_SET) during
            # the DMA ramp instead of stalling the first real exp.
            warm = out_pool.tile([128, 1], F32, name="act_warm", tag="warm")
            nc.vector.memset(warm[:], 0.0)
            nc.scalar.activation(warm[:], warm[:], Exp)
            # per-partition bias AP for the Madd slots' exp
            madd_bias = out_pool.tile([128, 1], F32, name="madd_bias", tag="mb")
            nc.gpsimd.memset(madd_bias[:], MADD_BIAS)
            # Warm the PE HAM clock gate during the DMA ramp: ~4us of dummy
            # matmuls flips the PE from K=4/8 (1.2 GHz) to 8/8 (2.4 GHz) so
            # the first real QKs run at full rate.
            warm_mm = out_pool.tile([128, 128], BF16, name="warm_mm", tag="wmm")
            nc.vector.memset(warm_mm[:], 0.0)
            warm_ps = psum_pool.tile(
                [128, 2, QCW], F32, name="warm_ps", tag="ps", bufs=3
            )
            for i in range(40):
                nc.tensor.matmul(
                    warm_ps[:, 0, 0:128],
                    lhsT=warm_mm[:],
                    rhs=warm_mm[:],
                    start=True,
                    stop=True,
                )

            # (1 - mask) transposed, contiguous per (qc, kp):
            # nmT[qc][kv%128, kp, h, qw] = 1 - mask[qc*512+qw, (2kp+h)*128+kv%128]
            # One tile per (qc, kp-half) so the first pair's TT only waits on
            # the chunk it reads, not the whole 8MB mask load.
            nmT_t = {}

            def load_nmT(qc, kh, split=1):
                t = nmT_pool.tile(
                    [128, KP // 2, 2, QCW], BF16, name=f"nmT_{qc}_{kh}"
                )
                k0 = kh * (KP // 2)
                for j in range(split):
                    w = (KP // 2) // split
                    nc.sync.dma_start(
                        t[:, j * w : (j + 1) * w],
                        m_d[:, qc, k0 + j * w : k0 + (j + 1) * w],
                    )
                nmT_t[(qc, kh)] = t

            def nmT(qc, kp):
                return nmT_t[(qc, kp // (KP // 2))][:, kp % (KP // 2)]

            # kT/qT split into chunk tiles so the first QK only waits on
            # the slice it reads (cuts the pipeline ramp at start).
            def load_kTh(p, h, eng=None):
                t = qkv_pool.tile(
                    [128, S // 2], BF16, name=f"kT_{p}_{h}", tag=f"kT{h}"
                )
                (eng or nc.sync).dma_start(
                    t[:], k_d[p, :, h * (S // 2) : (h + 1) * (S // 2)]
                )
                return t

            def load_qTc(p, c, eng=None):
                t = qkv_pool.tile(
                    [128, QCW], BF16, name=f"qT_{p}_{c}", tag=f"qT{c}"
                )
                (eng or nc.sync).dma_start(t[:], q_d[p, :, c * QCW : (c + 1) * QCW])
                return t

            def load_vb(p, eng=None):
                vb = qkv_pool.tile([128, T, D + 1], BF16, name=f"vb_{p}", tag="vb")
                (eng or nc.sync).dma_start(
                    vb[:, :, 0:D], v_d[p].rearrange("(t p) d -> p t d", p=128)
                )
                nc.gpsimd.memset(vb[:, :, D : D + 1], 1.0)
                return vb

            def load_pair(p):
                kTh = [load_kTh(p, h) for h in range(2)]
                qTc = [load_qTc(p, c) for c in range(NQC)]
                return qTc, kTh, load_vb(p)

            # Pair-0 DMAs issued in critical-path order: first QK needs
            # kT cols 0:128 + qT chunk 0; first TT needs mask chunk (0,0);
            # first EV needs vb. qT c0 goes first on sync (128KB) while kT
            # half 0 lands in four 256-col pieces on the scalar queue so the
            # first QK only waits on ~64KB there, not the whole 256KB tile.
            q0c0 = load_qTc(0, 0)
            k0h0 = qkv_pool.tile([128, S // 2], BF16, name="kT_0_0", tag="kT0")
            for j in range(4):
                nc.scalar.dma_start(
                    k0h0[:, j * 256 : (j + 1) * 256],
                    k_d[0, :, j * 256 : (j + 1) * 256],
                )
            k0h1 = load_kTh(0, 1, eng=nc.scalar)
            vb0 = load_vb(0, eng=nc.scalar)
            load_nmT(0, 0, split=4)
            idt = nmT_pool.tile([128, 128], BF16, name="ident")
            nc.scalar.dma_start(idt[:], i_d[:, :])
            q0rest = [load_qTc(0, c) for c in range(1, NQC)]
            load_nmT(1, 0, split=2)
            loads = {0: ([q0c0] + q0rest, [k0h0, k0h1], vb0)}
            loads[1] = load_pair(1)
            load_nmT(0, 1)
            load_nmT(1, 1)
            for qc in range(2, NQC):
                load_nmT(qc, 0)
                load_nmT(qc, 1)

            # Software-pipelined qc schedule. Each qc runs in two passes:
            #   P1(g):  QK (+Madd) matmuls, schrau affines, and exps — every
            #           op that READS an st2 PSUM bank, with no cross-engine
            #           producer in front of it. Bank release is paced by
            #           ACT's exp stream and the DVE affines at queue head.
            #   P2(g):  the TT mask-multiplies, EV matmuls, and the norm of
            #           g-1 — emitted AFTER P1(g+1), so on the DVE queue the
            #           next qc's affines always jump ahead of older TTs, and
            #           on the PE queue EVs never sit in front of QKs.
            # This breaks the convoy (aff waits TTs waits exps) that cost
            # ~2.4us/qc in the slot-interleaved schedule.
            step_prev = None  # per-slot deferred TT/EV of the previous qc
            fin_flush = None  # flushes prev qc's last EV + prev-prev's norm
            fin_prev = None   # norm+DMA closure of the lagged qc

            for p in range(pairs):
                qTc, kTh, vb = loads.pop(p)

                def kT_ap(kt, kTh=kTh):
                    return kTh[kt // (T // 2)][
                        :, (kt % (T // 2)) * 128 : (kt % (T // 2) + 1) * 128
                    ]
                if p + 2 < pairs:
                    loads[p + 2] = load_pair(p + 2)

                o_re = o_d[p].rearrange("(t p) d -> p t d", p=128)
                for qc in range(NQC):
                    # Two q-subtile outputs packed per PSUM bank (1040B,
                    # 8B-aligned 520B stride) so o_ps takes 2 banks total.
                    # Allocation is LAZY (first EV emission, in P2): the pool
                    # snapshots hazards at allocation, and the previous
                    # user's norm reads are emitted two P2s earlier.
                    o_cell = {}

                    def o_ps2(p=p, qc=qc, o_cell=o_cell):
                        if "t" not in o_cell:
                            o_cell["t"] = [
                                psum_pool.tile(
                                    [128, 2, D + 2], F32,
                                    name=f"ops_{p}_{qc}_{j}", tag="ops", bufs=2,
                                )
                                for j in range(NQS // 2)
                            ]
                        return o_cell["t"]

                    def o_ap(qs, o_ps2=o_ps2):
                        # [*, qs, 0:129] window; col 129 is padding.
                        return o_ps2()[qs // 2][:, qs % 2, 0 : D + 1]

                    def emit_ev(kp, em, o_ap=o_ap, vb=vb):
                        for h in (0, 1):
                            kt = 2 * kp + h
                            for qs in range(NQS):
                                # start clears has_written for the WHOLE bank;
                                # only the first qs of each shared bank may set
                                # it (the odd qs then overwrites its still-
                                # cleared region).
                                nc.tensor.matmul(
                                    o_ap(qs),
                                    lhsT=em[:, h, qs * 128 : (qs + 1) * 128],
                                    rhs=vb[:, kt, :],
                                    start=(kt == 0 and qs % 2 == 0),
                                    stop=(kt == T - 1),
                                    skip_group_check=True,
                                )

                    def finish_qc(o_ps2=o_ps2, p=p, qc=qc, o_re=o_re):
                        ot = o_ps2()
                        osb = out_pool.tile(
                            [128, NQS, D], F32, name=f"osb_{p}_{qc}", tag="osb"
                        )
                        for j in range(NQS // 2):
                            # batched: one reciprocal + one broadcast-multiply
                            # per PSUM bank (both packed q-subtiles at once)
                            rs2 = out_pool.tile(
                                [128, 2, 1], F32, name=f"rs_{p}_{qc}_{j}",
                                tag="rs", bufs=4,
                            )
                            nc.vector.reciprocal(
                                rs2[:], ot[j][:, :, D : D + 1]
                            )
                            nc.vector.tensor_tensor(
                                osb[:, 2 * j : 2 * j + 2, :],
                                ot[j][:, :, 0:D],
                                rs2[:].broadcast_to([128, 2, D]),
                                mult,
                            )
                        nc.sync.dma_start(
                            o_re[:, qc * NQS : (qc + 1) * NQS, :], osb[:]
                        )

                    # ---- P1: QK/Madd matmuls + bank-releasing reads,
                    #      interleaved with the PREVIOUS qc's deferred
                    #      TT/EV work (one slot's worth per kp) ----
                    ems = {}   # kp -> em tile (schrau/madd: final; TT: later)
                    e2s = {}   # kp -> e2 tile for TT slots
                    # pair-0 qc-0 runs all-TT: its P1 then depends only on
                    # q/k DMAs (exp needs no mask); the mask-consuming TT/EV
                    # work is deferred into the next qc's P1, by which time
                    # the Bnm chunks have landed. Avoids ~10us of ramp stall.
                    first_qc = p == 0 and qc == 0
                    for kp in range(KP):
                        is_madd = (qc, kp) in madd_set and not first_qc
                        st2 = psum_pool.tile(
                            [128, 2, QCW], F32, name=f"st_{p}_{qc}_{kp}",
                            tag="ps", bufs=3,
                        )
                        for h in (0, 1):
                            nc.tensor.matmul(
                                st2[:, h, :],
                                lhsT=kT_ap(2 * kp + h),
                                rhs=qTc[qc][:],
                                start=True,
                                stop=not is_madd,
                            )
                            if is_madd:
                                nc.tensor.matmul(
                                    st2[:, h, :],
                                    lhsT=idt[:],
                                    rhs=nmT(qc, kp)[:, h],
                                    start=False,
                                    stop=True,
                                    skip_group_check=True,
                                )
                        if (qc, kp) in schrau_set and not first_qc:
                            em = e_pool.tile(
                                [128, 2, QCW], BF16, name=f"em_{p}_{qc}_{kp}",
                                tag="em", bufs=16,
                            )
                            # single DVE op: bf16-bits Schraudolph with the
                            # mask folded in via the Bnm (=2^14*(1-m)) add
                            nc.vector.affine_then_add(
                                em[:].bitcast(mybir.dt.int16),
                                st2[:],
                                nmT(qc, kp),
                                SCHRAU_A16,
                                SCHRAU_C16,
                            )
                            ems[kp] = em
                        elif is_madd:
                            em = e_pool.tile(
                                [128, 2, QCW], BF16, name=f"em_{p}_{qc}_{kp}",
                                tag="em", bufs=16,
                            )
                            # mask already in the scores; bias closes the
                            # 2^14 bookkeeping. No DVE op.
                            nc.scalar.activation(
                                em[:], st2[:], Exp, scale=scale,
                                bias=madd_bias[:],
                            )
                            ems[kp] = em
                        else:
                            e2 = e_pool.tile(
                                [128, 2, QCW], BF16, name=f"e_{p}_{qc}_{kp}",
                                tag="e2", bufs=12,
                            )
                            nc.scalar.activation(e2[:], st2[:], Exp, scale=scale)
                            e2s[kp] = e2
                        if step_prev is not None:
                            step_prev(kp)
                    if step_prev is not None:
                        fin_prev = fin_flush(fin_prev)

                    def step_prev(kp, p=p, qc=qc, ems=ems, e2s=e2s,
                                  emit_ev=emit_ev):
                        # one deferred slot of THIS qc, run during the next
                        # qc's P1: the TT mask-multiply for slot kp and the
                        # EV batch for slot kp-1 (whose em is ready).
                        if kp in e2s:
                            em = e_pool.tile(
                                [128, 2, QCW], BF16,
                                name=f"em_{p}_{qc}_{kp}", tag="em", bufs=16,
                            )
                            nc.vector.tensor_tensor(
                                em[:], e2s[kp][:], nmT(qc, kp), mult
                            )
                            ems[kp] = em
                        if kp >= 1:
                            emit_ev(kp - 1, ems[kp - 1])

                    def fin_flush(fin_before, ems=ems, emit_ev=emit_ev,
                                  finish_qc=finish_qc):
                        # last EV batch of the lagged qc, then the norm+DMA
                        # of the qc before it (keeps norm behind the affines
                        # and TTs on the DVE queue).
                        emit_ev(KP - 1, ems[KP - 1])
                        if fin_before is not None:
                            fin_before()
                        return finish_qc

            # drain the pipeline tail: deferred TT/EV of the final qc, its
            # predecessor's norm, then the final norm+DMA
            for kp in range(KP):
                step_prev(kp)
            fin_prev = fin_flush(fin_prev)
            fin_prev()

    if split:
        nc.compile()
    return nc


_NC_CACHE = {}


def _get_nc(S=S_FULL, pairs=PAIRS):
    key = (S, pairs)
    if key not in _NC_CACHE:
        _NC_CACHE[key] = build_nc(S, pairs)
    return _NC_CACHE[key]


def kernel(q, k, v, mask):
    """Full-input entry point: q,k,v [4,16,2048,128] f32, mask [4,1,2048,2048]
    bool. Returns [4,16,2048,128] f32."""
    _install_ntff_hook()
    import ml_dtypes
    from concourse.bass_utils import run_bass_kernel_spmd

    bf16 = ml_dtypes.bfloat16
    # q, k pre-transposed per head: [B, H, D, S]; v kept [B, H, S, D]
    q = np.asarray(q, dtype=np.float32).transpose(0, 1, 3, 2).astype(bf16)
    k = np.asarray(k, dtype=np.float32).transpose(0, 1, 3, 2).astype(bf16)
    v = np.asarray(v, dtype=np.float32).astype(bf16)

    S = S_FULL
    NQC, KP, QCW = S // 512, (S // 128) // 2, 512
    # Bnm[b][p, qc, kp, h, qw] = 2^14 * (1 - mask[b, 0, qc*512+qw, (2kp+h)*128+p])
    # The 2^14 scale rides both the ACT path (multiplied into em) and the
    # Schraudolph path (added in bits-space); softmax normalization cancels it.
    nm = 16384.0 * (
        1.0 - np.asarray(mask).reshape(B, S, S).astype(np.float32)
    )
    # [b, qc, qw, kp, h, p] -> transpose to [b, p, qc, kp, h, qw]
    nmT = np.ascontiguousarray(
        nm.reshape(B, NQC, QCW, KP, 2, 128).transpose(0, 5, 1, 3, 4, 2)
    ).astype(bf16)

    ident = (0.0625 * np.eye(128, dtype=np.float32)).astype(bf16)
    hpc = H // (N_CORES // B)  # heads per core = 8
    in_maps = []
    for c in range(N_CORES):
        b = c // (N_CORES // B)
        h0 = (c % (N_CORES // B)) * hpc
        in_maps.append(
            {
                "q": np.ascontiguousarray(q[b, h0 : h0 + hpc]),
                "k": np.ascontiguousarray(k[b, h0 : h0 + hpc]),
                "v": np.ascontiguousarray(v[b, h0 : h0 + hpc]),
                "mask": nmT[b],
                "ident": ident,
            }
        )

    nc = _get_nc()
    trace = os.environ.get("BASS_ATTN_TRACE", "0") == "1"
    res = run_bass_kernel_spmd(nc, in_maps, list(range(N_CORES)), trace=trace)
    if trace:
        kernel.last_exec_time_ns = res.exec_time_ns
        kernel.last_results = res

    out = np.empty((B, H, S_FULL, D), dtype=np.float32)
    for c in range(N_CORES):
        b = c // (N_CORES // B)
        h0 = (c % (N_CORES // B)) * hpc
        out[b, h0 : h0 + hpc] = res.results[c]["o"]
    return out

